# revision 1
# baseline (speedup 1.0000x reference)
"""GQA attention kernel for Trainium2, tensor-parallel across 8 NeuronCores.

Problem: B=2, T=2048, D=2048, H=32 q-heads, G=8 kv-heads (GQA, rep=4), hd=64,
causal softmax attention + output projection, fp32 I/O.

Sharding (one KV group per core):
  core g: Wq[:, g*256:(g+1)*256], Wk/Wv[:, g*64:(g+1)*64], Wo[g*256:(g+1)*256, :]
  Each core computes its 4 heads' attention + partial output projection;
  host sums the 8 partial outputs (row-parallel Wo => partial-sum unshard).

On-device dataflow per core (all matmuls contract over the partition dim):
  QT = wq.T @ xT        [256, T]  (fp32r, full PE rate; 1/8 scale folded in)
  [KT; VT] = wkv.T @ xT [128, T]
  KT duplicated to partitions 64..127 so two heads' score matmuls run
  concurrently in disjoint PE row groups (contraction = hd = 64).
  ST_r = K @ QT_r       [128k, 512q] blocks (fp32r), causal block-skipped
  PT_r = exp(ST_r)      (ACT, bf16 out; diag blocks masked via affine_select)
  OT_r = [V|1].T @ PT_r [65, 512] (bf16, psum-accumulated over k tiles;
                         row 64 = softmax denominators)
  OT normalized by 1/denominator (DVE), stored as Wo lhsT layout.
  partial = OT.T @ wo   [t, 2048] (fp32r), DMA'd straight from PSUM to DRAM.
"""

import os
import sys

import numpy as np

for _p in ("/opt/trn_rl_repo", "/root/.axon_site/_ro/trn_rl_repo"):
    if os.path.isdir(_p) and _p not in sys.path:
        sys.path.insert(0, _p)

import concourse.bass as bass  # noqa: E402
import concourse.mybir as mybir  # noqa: E402
import concourse.tile as tile  # noqa: E402
from concourse import bacc  # noqa: E402
from concourse.bass_utils import run_bass_kernel_spmd  # noqa: E402
from concourse.masks import make_identity  # noqa: E402
from contextlib import ExitStack  # noqa: E402

B, T, D = 2, 2048, 2048
G, REP, HD = 8, 4, 64
DQ = REP * HD  # 256 q-dims per core
NCORES = 8
P = 128
TB = 512  # q/t block size
KO = D // P  # 16 contraction subtiles for projections
NT = T // TB  # 4 t-blocks
NKT = T // P  # 16 kpos tiles
F32 = mybir.dt.float32
F32R = mybir.dt.float32r
BF16 = mybir.dt.bfloat16
AF = mybir.ActivationFunctionType
SCALE = 1.0 / 8.0  # 1/sqrt(HD)


def r32(ap):
    return ap.bitcast(F32R)


def build_kernel(ctx, tc):
    nc = tc.nc
    xT = nc.dram_tensor("xT", [B, D, T], F32, kind="ExternalInput").ap()
    wq = nc.dram_tensor("wq", [D, DQ], F32, kind="ExternalInput").ap()
    wkv = nc.dram_tensor("wkv", [D, 2 * HD], F32, kind="ExternalInput").ap()
    wo = nc.dram_tensor("wo", [DQ, D], F32, kind="ExternalInput").ap()
    out = nc.dram_tensor("out", [B, T, D], F32, kind="ExternalOutput").ap()

    wpool = ctx.enter_context(tc.tile_pool(name="w", bufs=1))
    qt_pool = ctx.enter_context(tc.tile_pool(name="qt", bufs=2))
    kkt_pool = ctx.enter_context(tc.tile_pool(name="kkt", bufs=2))
    vt_pool = ctx.enter_context(tc.tile_pool(name="vt", bufs=2))
    v_pool = ctx.enter_context(tc.tile_pool(name="v", bufs=2))
    xt_pool = ctx.enter_context(tc.tile_pool(name="xt", bufs=6))
    p_pool = ctx.enter_context(tc.tile_pool(name="p", bufs=3))
    o_pool = ctx.enter_context(tc.tile_pool(name="ot", bufs=2))
    r_pool = ctx.enter_context(tc.tile_pool(name="rcp", bufs=3))
    pp = ctx.enter_context(tc.tile_pool(name="pp", bufs=2, space="PSUM"))
    sp = pp
    op = pp
    wp = pp

    # persistent weights
    wq_sb = wpool.tile([P, KO, DQ], F32R, tag="wq")
    nc.gpsimd.dma_start(wq_sb[:], wq.rearrange("(ko p) m -> p ko m", p=P))
    wkv_sb = wpool.tile([P, KO, 2 * HD], F32R, tag="wkv")
    nc.gpsimd.dma_start(wkv_sb[:], wkv.rearrange("(ko p) m -> p ko m", p=P))
    wo_sb = wpool.tile([P, DQ // P, D], F32R, tag="wo")
    nc.gpsimd.dma_start(wo_sb[:], wo.rearrange("(ko p) m -> p ko m", p=P))
    ident = wpool.tile([P, P], F32, tag="ident")
    make_identity(nc, ident[:])

    for b in range(B):
        # ---------------- projections ----------------
        qt_sb = qt_pool.tile([P, 2, T], F32R, tag="qt")  # QT, scaled by 1/8
        kkt_sb = kkt_pool.tile([P, T], F32R, tag="kkt")  # KT duplicated twice
        vt_sb = vt_pool.tile([P, T], F32, tag="vt")  # VT on partitions 64..127
        for tb in range(NT):
            q_ps0 = pp.tile([P, TB], F32, tag="A")
            q_ps1 = pp.tile([P, TB], F32, tag="B")
            kv_ps = pp.tile([P, TB], F32, tag="C")
            for ko in range(KO):
                xt = xt_pool.tile([P, TB], F32R, tag="xt")
                nc.gpsimd.dma_start(
                    xt[:], xT[b, ko * P : (ko + 1) * P, tb * TB : (tb + 1) * TB]
                )
                st, sp_ = (ko == 0), (ko == KO - 1)
                nc.tensor.matmul(
                    q_ps0[:], wq_sb[:, ko, 0:P], xt[:], start=st, stop=sp_
                )
                nc.tensor.matmul(
                    q_ps1[:], wq_sb[:, ko, P:DQ], xt[:], start=st, stop=sp_
                )
                nc.tensor.matmul(
                    kv_ps[:], wkv_sb[:, ko, :], xt[:], start=st, stop=sp_
                )
            ts = slice(tb * TB, (tb + 1) * TB)
            nc.scalar.activation(qt_sb[:, 0, ts], q_ps0[:], AF.Copy, scale=SCALE)
            nc.scalar.activation(qt_sb[:, 1, ts], q_ps1[:], AF.Copy, scale=SCALE)
            nc.vector.tensor_copy(kkt_sb[0:HD, ts], kv_ps[0:HD, :])
            nc.vector.tensor_copy(vt_sb[HD:P, ts], kv_ps[HD:P, :])
            # duplicate KT to partitions 64..127 (SBUF->SBUF DMA moves partitions)
            nc.sync.dma_start(kkt_sb[HD:P, ts], kkt_sb[0:HD, ts])

        # ---------------- V transpose -> [kpos, hd|1] bf16 ----------------
        v1_sb = v_pool.tile([P, NKT, HD + 1], BF16, tag="v1")
        nc.gpsimd.memset(v1_sb[:, :, HD : HD + 1], 1.0)
        for kt in range(NKT):
            tr_ps = wp.tile([P, TB], F32, tag="D")
            nc.tensor.transpose(
                tr_ps[:, 0:HD],
                vt_sb[HD:P, kt * P : (kt + 1) * P],
                ident[HD:P, HD:P],
            )
            nc.vector.tensor_copy(v1_sb[:, kt, 0:HD], tr_ps[:, 0:HD])

        # ---------------- attention + output proj, per q-block ----------------
        for qb in range(NT):
            qs = slice(qb * TB, (qb + 1) * TB)
            nkt = 4 * (qb + 1)  # causal: kpos tiles 0..nkt-1
            ot_sb = o_pool.tile([P, 2, TB], F32R, tag="ot")
            for pair in range(2):
                o_ps = []
                for i in range(2):
                    o_ps_i = op.tile([P, TB], F32, tag="C", name=f"o_ps_{i}")
                    o_ps.append(o_ps_i)
                for kt in range(nkt):
                    ks = slice(kt * P, (kt + 1) * P)
                    s_ps0 = sp.tile([P, TB], F32, tag="A")
                    s_ps1 = sp.tile([P, TB], F32, tag="B")
                    nc.tensor.matmul(
                        s_ps0[:],
                        kkt_sb[0:HD, ks],
                        qt_sb[0:HD, pair, qs],
                        start=True,
                        stop=True,
                        tile_position=(0, 0),
                    )
                    nc.tensor.matmul(
                        s_ps1[:],
                        kkt_sb[HD:P, ks],
                        qt_sb[HD:P, pair, qs],
                        start=True,
                        stop=True,
                        tile_position=(64, 0),
                    )
                    pt0 = p_pool.tile([P, TB], BF16, tag="p0")
                    pt1 = p_pool.tile([P, TB], BF16, tag="p1")
                    nc.scalar.activation(pt0[:], s_ps0[:], AF.Exp)
                    nc.scalar.activation(pt1[:], s_ps1[:], AF.Exp)
                    if kt >= qb * 4:  # diagonal block: causal mask
                        for pt in (pt0, pt1):
                            nc.gpsimd.affine_select(
                                out=pt[:],
                                in_=pt[:],
                                compare_op=mybir.AluOpType.is_ge,
                                fill=0.0,
                                base=qb * TB - kt * P,
                                channel_multiplier=-1,
                                pattern=[[1, TB]],
                            )
                    st, sp_ = (kt == 0), (kt == nkt - 1)
                    nc.tensor.matmul(
                        o_ps[0][0 : HD + 1, :], v1_sb[:, kt, :], pt0[:],
                        start=st, stop=sp_,
                    )
                    nc.tensor.matmul(
                        o_ps[1][0 : HD + 1, :], v1_sb[:, kt, :], pt1[:],
                        start=st, stop=sp_,
                    )
                # normalize: ot[r] = o_ps[r][:64] / o_ps[r][64]
                for i in range(2):
                    sums = r_pool.tile([1, TB], F32, tag="sums")
                    nc.vector.tensor_copy(sums[:], o_ps[i][HD : HD + 1, :])
                    rb = r_pool.tile([HD, TB], F32, tag="rb")
                    nc.gpsimd.partition_broadcast(rb[:], sums[:])
                    nc.vector.reciprocal(rb[:], rb[:])
                    nc.vector.tensor_mul(
                        ot_sb[i * HD : (i + 1) * HD, pair, :],
                        o_ps[i][0:HD, :],
                        rb[:],
                    )
            # Wo partial for this q-block's 512 tokens
            for tt in range(4):
                rows = slice(qb * TB + tt * P, qb * TB + (tt + 1) * P)
                lslice = slice(tt * P, (tt + 1) * P)
                for nb in range(4):
                    wo_ps = wp.tile([P, TB], F32, tag="D")
                    for ko in range(2):
                        nc.tensor.matmul(
                            wo_ps[:],
                            ot_sb[:, ko, lslice],
                            wo_sb[:, ko, nb * TB : (nb + 1) * TB],
                            start=(ko == 0),
                            stop=(ko == 1),
                        )
                    stg = p_pool.tile([P, TB], F32, tag="stg")
                    nc.vector.tensor_copy(stg[:], wo_ps[:])
                    nc.sync.dma_start(out[b, rows, nb * TB : (nb + 1) * TB], stg[:])


_NC_CACHE = {}


def get_nc():
    if "nc" not in _NC_CACHE:
        nc = bacc.Bacc("TRN2", target_bir_lowering=False, debug=False)
        with tile.TileContext(nc) as tc, ExitStack() as ctx:
            build_kernel(ctx, tc)
        nc.compile()
        _NC_CACHE["nc"] = nc
    return _NC_CACHE["nc"]


def make_in_maps(x, Wq, Wk, Wv, Wo):
    xT = np.ascontiguousarray(np.transpose(np.asarray(x, np.float32), (0, 2, 1)))
    Wq, Wk, Wv, Wo = (np.asarray(w, np.float32) for w in (Wq, Wk, Wv, Wo))
    in_maps = []
    for g in range(NCORES):
        in_maps.append(
            {
                "xT": xT,
                "wq": np.ascontiguousarray(Wq[:, g * DQ : (g + 1) * DQ]),
                "wkv": np.ascontiguousarray(
                    np.concatenate(
                        [Wk[:, g * HD : (g + 1) * HD], Wv[:, g * HD : (g + 1) * HD]],
                        axis=1,
                    )
                ),
                "wo": np.ascontiguousarray(Wo[g * DQ : (g + 1) * DQ, :]),
            }
        )
    return in_maps


def run(x, Wq, Wk, Wv, Wo, trace=False):
    nc = get_nc()
    in_maps = make_in_maps(x, Wq, Wk, Wv, Wo)
    res = run_bass_kernel_spmd(nc, in_maps, list(range(NCORES)), trace=trace)
    acc = np.zeros((B, T, D), np.float32)
    for r in res.results:
        acc += r["out"]
    return acc, res


def kernel(x, Wq, Wk, Wv, Wo):
    return run(x, Wq, Wk, Wv, Wo)[0]



# revision 27
# speedup vs baseline: 222.9120x; 222.9120x over previous
"""GQA attention kernel for Trainium2, tensor-parallel across 8 NeuronCores.

Problem: B=2, T=2048, D=2048, H=32 q-heads, G=8 kv-heads (GQA, rep=4), hd=64,
causal softmax attention + output projection, fp32 I/O.

Sharding (one KV group per core):
  core g: Wq[:, g*256:(g+1)*256], Wk/Wv[:, g*64:(g+1)*64], Wo[g*256:(g+1)*256, :]
  Each core computes its 4 heads' attention + partial output projection;
  host sums the 8 partial outputs (row-parallel Wo => partial-sum unshard).

v3 layout (bf16 activations/weights on device; fp32 PSUM accumulation):
  QT = wq.T @ xT        [256, T]  (1/8 scale folded into the PSUM->SBUF copy)
  [KT; VT] = wkv.T @ xT [128, T]; KT duplicated to partitions 64..127 so two
  heads' score matmuls run in disjoint PE row groups (contraction = hd = 64).
  Scores for a head-pair land in one 2-bank PSUM tile [128, 2, 512] so exp and
  the diagonal causal mask run once per pair (half the ACT/Pool instructions).
  Diagonal blocks are trapezoid-trimmed: matmul/exp/mask/PV only touch
  columns >= 128*j of the block (j = diagonal sub-tile index).
  V tiles carry 64 columns of ones -> PV matmul replicates the softmax
  denominator across partitions 64..127; normalize = DVE reciprocal + mul.
  partial = OT.T @ wo, staged per 128-token chunk to SBUF, DMA'd fp32; Wo
  work for block i is interleaved into block i+1's kt loop to fill PE stalls.
"""

import os
import sys

import numpy as np

for _p in ("/opt/trn_rl_repo", "/root/.axon_site/_ro/trn_rl_repo"):
    if os.path.isdir(_p) and _p not in sys.path:
        sys.path.insert(0, _p)

import concourse.bass as bass  # noqa: E402
import concourse.mybir as mybir  # noqa: E402
import concourse.tile as tile  # noqa: E402
from concourse import bacc  # noqa: E402
from concourse.bass_utils import run_bass_kernel_spmd  # noqa: E402
from concourse.masks import make_identity  # noqa: E402
from contextlib import ExitStack  # noqa: E402

B, T, D = 2, 2048, 2048
G, REP, HD = 8, 4, 64
DQ = REP * HD  # 256 q-dims per core
NCORES = 8
P = 128
TB = 512  # q/t block size
KO = D // P  # 16 contraction subtiles for projections
NT = T // TB  # 4 t-blocks
NKT = T // P  # 16 kpos tiles
F32 = mybir.dt.float32
BF16 = mybir.dt.bfloat16
AF = mybir.ActivationFunctionType
SCALE = 1.0 / 8.0  # 1/sqrt(HD)


def build_kernel(ctx, tc):
    nc = tc.nc
    # host pre-reshapes everything into the exact SBUF layouts
    xT = nc.dram_tensor("xT", [B, P, KO, T], BF16, kind="ExternalInput").ap()
    wq = nc.dram_tensor("wq", [P, KO, DQ], BF16, kind="ExternalInput").ap()
    wkv = nc.dram_tensor("wkv", [P, KO, 2 * HD], BF16, kind="ExternalInput").ap()
    wo = nc.dram_tensor("wo", [P, 2, D], BF16, kind="ExternalInput").ap()
    # out[b, qb, p, tt, :] = partial for token qb*512 + tt*128 + p
    out = nc.dram_tensor("out", [B, NT, P, 4, D], F32, kind="ExternalOutput").ap()

    wpool = ctx.enter_context(tc.tile_pool(name="w", bufs=1))
    qt_pool = ctx.enter_context(tc.tile_pool(name="qt", bufs=2))
    kkt_pool = ctx.enter_context(tc.tile_pool(name="kkt", bufs=2))
    vt_pool = ctx.enter_context(tc.tile_pool(name="vt", bufs=2))
    v1_pool = ctx.enter_context(tc.tile_pool(name="v1", bufs=2))
    xt_pool = ctx.enter_context(tc.tile_pool(name="xt", bufs=3))
    p_pool = ctx.enter_context(tc.tile_pool(name="p", bufs=4))
    o_pool = ctx.enter_context(tc.tile_pool(name="ot", bufs=2))
    r_pool = ctx.enter_context(tc.tile_pool(name="rcp", bufs=3))
    stg_pool = ctx.enter_context(tc.tile_pool(name="stg", bufs=4))
    pp = ctx.enter_context(tc.tile_pool(name="pp", bufs=2, space="PSUM"))

    # persistent weights; wq/wkv split so the first projection matmuls start
    # after ~1/4 of the weight traffic; wo deferred until after projections
    wq_sb = wpool.tile([P, KO, DQ], BF16, tag="wq")
    nc.scalar.dma_start(wq_sb[:, 0:4, :], wq[:, 0:4, :])
    wkv_sb = wpool.tile([P, KO, 2 * HD], BF16, tag="wkv")

    # all x tiles prefetched up-front (pool recycling paces the later ones);
    # the first tile arrives in ko quarters interleaved with the weight
    # chunks so the DMA device delivers ko 0..3 of everything first
    xts = {}
    for b in range(B):
        for tb in range(NT):
            xts[(b, tb)] = xt_pool.tile([P, KO, TB], BF16, tag="xt", name=f"xt{b}{tb}")
    src00 = xT[0, :, :, 0:TB]
    nc.scalar.dma_start(xts[(0, 0)][:, 0:4, :], src00[:, 0:4, :])
    nc.scalar.dma_start(wq_sb[:, 4:KO, :], wq[:, 4:KO, :])
    nc.scalar.dma_start(xts[(0, 0)][:, 4:KO, :], src00[:, 4:KO, :])
    nc.scalar.dma_start(wkv_sb[:], wkv[:])
    ident = wpool.tile([P, P], BF16, tag="ident")
    make_identity(nc, ident[:])
    for b in range(B):
        for tb in range(NT):
            if (b, tb) == (0, 0):
                continue
            nc.scalar.dma_start(
                xts[(b, tb)][:], xT[b, :, :, tb * TB : (tb + 1) * TB]
            )

    qts, kkts, vts, v1s = {}, {}, {}, {}
    for b in range(B):
        qts[b] = qt_pool.tile([P, 2, T], BF16, tag="qt", name=f"qt{b}")
        kkts[b] = kkt_pool.tile([P, T], BF16, tag="kkt", name=f"kkt{b}")
        vts[b] = vt_pool.tile([P, T], BF16, tag="vt", name=f"vt{b}")
        v1s[b] = v1_pool.tile([P, NKT, P], BF16, tag="v1", name=f"v1{b}")

    # ---------------- projections ----------------
    for b in range(B):
        qt_sb, kkt_sb, vt_sb, v1_sb = qts[b], kkts[b], vts[b], v1s[b]
        for tb in range(NT):
            xt = xts[(b, tb)]
            q_ps = pp.tile([P, 2, TB], F32, tag="S")
            kv_ps = pp.tile([P, TB], F32, tag="C")
            # for the very first block, run all q matmuls before the kv ones:
            # wq + x alone give PE a full runway while wkv streams in
            if b == 0 and tb == 0:
                for ko in range(KO):
                    st, sp_ = (ko == 0), (ko == KO - 1)
                    nc.tensor.matmul(
                        q_ps[:, 0, :], wq_sb[:, ko, 0:P], xt[:, ko, :],
                        start=st, stop=sp_,
                    )
                    nc.tensor.matmul(
                        q_ps[:, 1, :], wq_sb[:, ko, P:DQ], xt[:, ko, :],
                        start=st, stop=sp_,
                    )
                for ko in range(KO):
                    nc.tensor.matmul(
                        kv_ps[:], wkv_sb[:, ko, :], xt[:, ko, :],
                        start=(ko == 0), stop=(ko == KO - 1),
                    )
            else:
                for ko in range(KO):
                    st, sp_ = (ko == 0), (ko == KO - 1)
                    nc.tensor.matmul(
                        q_ps[:, 0, :], wq_sb[:, ko, 0:P], xt[:, ko, :],
                        start=st, stop=sp_,
                    )
                    nc.tensor.matmul(
                        q_ps[:, 1, :], wq_sb[:, ko, P:DQ], xt[:, ko, :],
                        start=st, stop=sp_,
                    )
                    nc.tensor.matmul(
                        kv_ps[:], wkv_sb[:, ko, :], xt[:, ko, :], start=st, stop=sp_
                    )
            ts_ = slice(tb * TB, (tb + 1) * TB)
            nc.vector.tensor_copy(kkt_sb[0:HD, ts_], kv_ps[0:HD, :])
            nc.vector.tensor_copy(vt_sb[HD:P, ts_], kv_ps[HD:P, :])
            nc.vector.tensor_scalar_mul(qt_sb[:, :, ts_], q_ps[:], SCALE)
            # duplicate KT to partitions 64..127 (SBUF->SBUF DMA moves partitions)
            nc.sync.dma_start(kkt_sb[HD:P, ts_], kkt_sb[0:HD, ts_])

        # V transpose -> v1[kpos, 0:64]; ones in v1[kpos, 64:128] replicate the
        # softmax denominator across partitions 64..127 of the PV output
        nc.gpsimd.memset(v1_sb[:, :, HD:P], 1.0)
        for kt in range(NKT):
            tr_ps = pp.tile([P, HD], BF16, tag="D")
            nc.tensor.transpose(
                tr_ps[:],
                vt_sb[HD:P, kt * P : (kt + 1) * P],
                ident[HD:P, HD:P],
            )
            nc.vector.tensor_copy(v1_sb[:, kt, 0:HD], tr_ps[:])

    wo_sb = wpool.tile([P, 2, D], BF16, tag="wo")
    nc.scalar.dma_start(wo_sb[:], wo[:])

    # ---------------- attention + output proj ----------------
    # Wo work of the previous block drips into the current kt loop (one chunk
    # per kt iteration) so it fills PE stalls caused by the exp chain.
    pending = []

    def emit_pending(n):
        for _ in range(min(n, len(pending))):
            pending.pop(0)()

    def queue_wo(b, qb, ot_sb, final=False):
        state = {}
        if final:
            # tail flush: 2-bank S-pool tiles (free after the last kt loop),
            # merged ACT/DVE-alternating copies, per-half stores -> the flush
            # runs matmul-dense and the last bytes leave right behind it
            for tt in range(4):
                for nbp in range(2):

                    def chunk2(tt=tt, nbp=nbp):
                        if nbp == 0:
                            state[tt] = stg_pool.tile(
                                [P, D], F32, tag="stg", name=f"stg{b}{qb}{tt}"
                            )
                        stg = state[tt]
                        wo_ps2 = pp.tile([P, 2, TB], F32, tag="S", name="wo_ps2")
                        for sub in range(2):
                            nb = 2 * nbp + sub
                            for ko in range(2):
                                nc.tensor.matmul(
                                    wo_ps2[:, sub, :],
                                    ot_sb[:, ko, tt * P : (tt + 1) * P],
                                    wo_sb[:, ko, nb * TB : (nb + 1) * TB],
                                    start=(ko == 0),
                                    stop=(ko == 1),
                                )
                        hs = slice(2 * nbp * TB, (2 * nbp + 2) * TB)
                        if (tt * 2 + nbp) % 2 == 0:
                            nc.scalar.activation(stg[:, hs], wo_ps2[:], AF.Copy)
                        else:
                            nc.vector.tensor_copy(stg[:, hs], wo_ps2[:])
                        nc.sync.dma_start(out[b, qb, :, tt, hs], stg[:, hs])

                    pending.append(chunk2)
            return
        for tt in range(4):
            for nb in range(4):

                def chunk(tt=tt, nb=nb):
                    if nb == 0:
                        state[tt] = stg_pool.tile(
                            [P, D], F32, tag="stg", name=f"stg{b}{qb}{tt}"
                        )
                    stg = state[tt]
                    wo_ps = pp.tile([P, TB], F32, tag="D")
                    for ko in range(2):
                        nc.tensor.matmul(
                            wo_ps[:],
                            ot_sb[:, ko, tt * P : (tt + 1) * P],
                            wo_sb[:, ko, nb * TB : (nb + 1) * TB],
                            start=(ko == 0),
                            stop=(ko == 1),
                        )
                    ns_ = slice(nb * TB, (nb + 1) * TB)
                    nc.vector.tensor_copy(stg[:, ns_], wo_ps[:])
                    if nb == 3:
                        nc.sync.dma_start(out[b, qb, :, tt, :], stg[:])

                pending.append(chunk)

    for b in range(B):
        qt_sb, kkt_sb, v1_sb = qts[b], kkts[b], v1s[b]
        # largest first; the all-diagonal qb=0 sits second-to-last so its long
        # per-tile chains are filled by qb=2's Wo drip, and the run ends on
        # qb=1 whose trailing off-diagonal tiles have short chains
        for qb in (3, 2, 0, 1):
            nkt = 4 * (qb + 1)
            # drip the previous block's Wo chunks evenly across this block's
            # kt iterations so they fill exp-chain PE stalls
            emit_rate = len(pending) / (2.0 * nkt)
            emit_acc = 0.0
            emitted = 0
            ot_sb = o_pool.tile([P, 2, TB], BF16, tag="ot")
            # diagonal blocks first: their long S->exp->mask->PV chains hide
            # behind the off-diagonal stream that follows (the j=0 diagonal
            # covers the full q range, so it can carry start=True)
            kt_order = list(range(4 * qb, nkt)) + list(range(0, 4 * qb))
            for pair in range(2):
                o_ps = []
                for i in range(2):
                    o_ps.append(pp.tile([P, TB], F32, tag="C", name=f"o_ps_{i}"))
                for ki, kt in enumerate(kt_order):
                    ks = slice(kt * P, (kt + 1) * P)
                    j = kt - 4 * qb  # >=0 on diagonal blocks
                    cs = slice(128 * j, TB) if j >= 0 else slice(0, TB)
                    w = TB - cs.start
                    qs = slice(qb * TB + cs.start, (qb + 1) * TB)
                    s_ps = pp.tile([P, 2, TB], F32, tag="S")
                    nc.tensor.matmul(
                        s_ps[:, 0, cs],
                        kkt_sb[0:HD, ks],
                        qt_sb[0:HD, pair, qs],
                        start=True,
                        stop=True,
                        tile_position=(0, 0),
                    )
                    nc.tensor.matmul(
                        s_ps[:, 1, cs],
                        kkt_sb[HD:P, ks],
                        qt_sb[HD:P, pair, qs],
                        start=True,
                        stop=True,
                        tile_position=(64, 0),
                    )
                    pt = p_pool.tile([P, 2, TB], BF16, tag="p")
                    nc.scalar.activation(pt[:, :, cs], s_ps[:, :, cs], AF.Exp)
                    if j >= 0:  # diagonal: zero out q < k inside the block
                        nc.gpsimd.affine_select(
                            out=pt[:, :, cs],
                            in_=pt[:, :, cs],
                            compare_op=mybir.AluOpType.is_ge,
                            fill=0.0,
                            base=0,
                            channel_multiplier=-1,
                            pattern=[[0, 2], [1, w]],
                        )
                    st, sp_ = (ki == 0), (ki == nkt - 1)
                    nc.tensor.matmul(
                        o_ps[0][:, cs], v1_sb[:, kt, :], pt[:, 0, cs],
                        start=st, stop=sp_,
                    )
                    nc.tensor.matmul(
                        o_ps[1][:, cs], v1_sb[:, kt, :], pt[:, 1, cs],
                        start=st, stop=sp_,
                    )
                    emit_acc += emit_rate
                    n_emit = int(emit_acc) - emitted
                    emitted += n_emit
                    emit_pending(n_emit)
                if b == B - 1 and qb == 1 and pair == 1:
                    # very last pair: normalize per 128-token chunk straight
                    # from PSUM so the first tail-flush Wo matmul starts after
                    # ~1us instead of after the full-width chain
                    rbs = []
                    for i in range(2):
                        rb = r_pool.tile([HD, TB], F32, tag="rb", name=f"rbf{i}")
                        if i == 0:
                            nc.vector.reciprocal(rb[:], o_ps[i][HD:P, :])
                        else:
                            nc.scalar.activation(
                                rb[:], o_ps[i][HD:P, :], AF.Reciprocal
                            )
                        rbs.append(rb)
                    for tt in range(4):
                        tsl = slice(tt * P, (tt + 1) * P)
                        for i in range(2):
                            nc.vector.tensor_mul(
                                ot_sb[i * HD : (i + 1) * HD, pair, tsl],
                                o_ps[i][0:HD, tsl],
                                rbs[i][:, tsl],
                            )
                    continue
                # evacuate PSUM with one ACT copy per head (frees the o_ps
                # banks for the next pair ~1.5us earlier than recip+mul would),
                # then normalize: ot[r] = o_cp[0:64] / denom (rows 64:127)
                for i in range(2):
                    o_cp = r_pool.tile([P, TB], F32, tag="ocp", name=f"ocp{i}")
                    if i == 0:  # both banks evacuate in parallel (DVE + ACT)
                        nc.vector.tensor_copy(o_cp[:], o_ps[i][:])
                    else:
                        nc.scalar.activation(o_cp[:], o_ps[i][:], AF.Copy)
                    rb = r_pool.tile([HD, TB], F32, tag="rb")
                    nc.vector.reciprocal(rb[:], o_cp[HD:P, :])
                    nc.vector.tensor_mul(
                        ot_sb[i * HD : (i + 1) * HD, pair, :],
                        o_cp[0:HD, :],
                        rb[:],
                    )
            queue_wo(b, qb, ot_sb, final=(b == B - 1 and qb == 1))
    emit_pending(len(pending))


_NC_CACHE = {}


def get_nc():
    if "nc" not in _NC_CACHE:
        nc = bacc.Bacc("TRN2", target_bir_lowering=False, debug=False)
        with tile.TileContext(nc) as tc, ExitStack() as ctx:
            build_kernel(ctx, tc)
        nc.compile()
        _NC_CACHE["nc"] = nc
    return _NC_CACHE["nc"]


def make_in_maps(x, Wq, Wk, Wv, Wo):
    import ml_dtypes

    bf = ml_dtypes.bfloat16
    x = np.asarray(x, np.float32)
    # xT[b, p, ko, t] = x[b, t, ko*128 + p]
    xTr = np.ascontiguousarray(
        x.transpose(0, 2, 1).reshape(B, KO, P, T).transpose(0, 2, 1, 3)
    ).astype(bf)
    Wq, Wk, Wv, Wo = (np.asarray(w, np.float32) for w in (Wq, Wk, Wv, Wo))
    in_maps = []
    for g in range(NCORES):
        wq_g = Wq[:, g * DQ : (g + 1) * DQ].reshape(KO, P, DQ).transpose(1, 0, 2)
        wkv_g = (
            np.concatenate(
                [Wk[:, g * HD : (g + 1) * HD], Wv[:, g * HD : (g + 1) * HD]], axis=1
            )
            .reshape(KO, P, 2 * HD)
            .transpose(1, 0, 2)
        )
        wo_g = Wo[g * DQ : (g + 1) * DQ, :].reshape(2, P, D).transpose(1, 0, 2)
        in_maps.append(
            {
                "xT": xTr,
                "wq": np.ascontiguousarray(wq_g).astype(bf),
                "wkv": np.ascontiguousarray(wkv_g).astype(bf),
                "wo": np.ascontiguousarray(wo_g).astype(bf),
            }
        )
    return in_maps


def run(x, Wq, Wk, Wv, Wo, trace=False):
    nc = get_nc()
    in_maps = make_in_maps(x, Wq, Wk, Wv, Wo)
    res = run_bass_kernel_spmd(nc, in_maps, list(range(NCORES)), trace=trace)
    acc = np.zeros((B, NT, P, 4, D), np.float32)
    for r in res.results:
        acc += np.asarray(r["out"], np.float32)
    # [b, qb, p, tt, d] -> [b, qb, tt, p, d] -> [B, T, D]
    full = np.ascontiguousarray(acc.transpose(0, 1, 3, 2, 4)).reshape(B, T, D)
    return full, res


def kernel(x, Wq, Wk, Wv, Wo):
    return run(x, Wq, Wk, Wv, Wo)[0]


# revision 34
# speedup vs baseline: 230.2895x; 1.0331x over previous
"""GQA attention kernel for Trainium2, tensor-parallel across 8 NeuronCores.

Problem: B=2, T=2048, D=2048, H=32 q-heads, G=8 kv-heads (GQA, rep=4), hd=64,
causal softmax attention + output projection, fp32 I/O.

Sharding (one KV group per core):
  core g: Wq[:, g*256:(g+1)*256], Wk/Wv[:, g*64:(g+1)*64], Wo[g*256:(g+1)*256, :]
  Each core computes its 4 heads' attention + partial output projection;
  host sums the 8 partial outputs (row-parallel Wo => partial-sum unshard).

Device dataflow (bf16 activations/weights/partials; fp32 PSUM accumulation):
  QT = wq.T @ xT        [256, T]  (1/8 scale folded into the PSUM->SBUF copy)
  [KT; VT] = wkv.T @ xT [128, T]; KT duplicated to partitions 64..127 so two
  heads' score matmuls run in disjoint PE row groups (contraction = hd = 64).
  Scores for a head-pair land in one 2-bank PSUM tile [128, 2, 512] so exp and
  the diagonal causal mask run once per pair (half the ACT/Pool instructions).
  Diagonal blocks are trapezoid-trimmed: matmul/exp/mask/PV only touch
  columns >= 128*j of the block (j = diagonal sub-tile index), and are
  processed first within a block so their long chains hide behind the
  off-diagonal stream (except the very first block, which needs the last
  projection tile and therefore runs ascending).
  V tiles carry 64 columns of ones -> the PV matmul replicates the softmax
  denominator across partitions 64..127; normalize = reciprocal + mul off an
  ACT/DVE PSUM evacuation copy (frees the o banks for the next head pair).
  partial = OT.T @ wo, staged per 128-token chunk to SBUF as bf16, stored per
  qb/tt block; host upcasts and sums the 8 partials. Wo work for block i is
  dripped into block i+1's kt loop (one chunk per iteration) to fill the PE
  stalls left by the exp chain; the last block flushes through 2-bank PSUM
  tiles with per-half stores so the tail is matmul-dense.
"""

import os
import sys

import numpy as np

for _p in ("/opt/trn_rl_repo", "/root/.axon_site/_ro/trn_rl_repo"):
    if os.path.isdir(_p) and _p not in sys.path:
        sys.path.insert(0, _p)

import concourse.bass as bass  # noqa: E402
import concourse.mybir as mybir  # noqa: E402
import concourse.tile as tile  # noqa: E402
from concourse import bacc  # noqa: E402
from concourse.bass_utils import run_bass_kernel_spmd  # noqa: E402
from concourse.masks import make_identity  # noqa: E402
from contextlib import ExitStack  # noqa: E402

B, T, D = 2, 2048, 2048
G, REP, HD = 8, 4, 64
DQ = REP * HD  # 256 q-dims per core
NCORES = 8
P = 128
TB = 512  # q/t block size
KO = D // P  # 16 contraction subtiles for projections
NT = T // TB  # 4 t-blocks
NKT = T // P  # 16 kpos tiles
F32 = mybir.dt.float32
BF16 = mybir.dt.bfloat16
AF = mybir.ActivationFunctionType
SCALE = 1.0 / 8.0  # 1/sqrt(HD)


def build_kernel(ctx, tc):
    nc = tc.nc
    # host pre-reshapes everything into the exact SBUF layouts
    xT = nc.dram_tensor("xT", [B, P, KO, T], BF16, kind="ExternalInput").ap()
    wq = nc.dram_tensor("wq", [P, KO, DQ], BF16, kind="ExternalInput").ap()
    wkv = nc.dram_tensor("wkv", [P, KO, 2 * HD], BF16, kind="ExternalInput").ap()
    wo = nc.dram_tensor("wo", [P, 2, D], BF16, kind="ExternalInput").ap()
    # out[b, qb, p, tt, :] = partial for token qb*512 + tt*128 + p
    out = nc.dram_tensor("out", [B, NT, P, 4, D], BF16, kind="ExternalOutput").ap()

    wpool = ctx.enter_context(tc.tile_pool(name="w", bufs=1))
    qt_pool = ctx.enter_context(tc.tile_pool(name="qt", bufs=2))
    kkt_pool = ctx.enter_context(tc.tile_pool(name="kkt", bufs=2))
    vt_pool = ctx.enter_context(tc.tile_pool(name="vt", bufs=2))
    v1_pool = ctx.enter_context(tc.tile_pool(name="v1", bufs=2))
    xt_pool = ctx.enter_context(tc.tile_pool(name="xt", bufs=3))
    p_pool = ctx.enter_context(tc.tile_pool(name="p", bufs=6))
    o_pool = ctx.enter_context(tc.tile_pool(name="ot", bufs=2))
    r_pool = ctx.enter_context(tc.tile_pool(name="rcp", bufs=3))
    stg_pool = ctx.enter_context(tc.tile_pool(name="stg", bufs=4))
    pp = ctx.enter_context(tc.tile_pool(name="pp", bufs=2, space="PSUM"))

    # persistent weights; wq/wkv split so the first projection matmuls start
    # after ~1/4 of the weight traffic; wo deferred until after projections
    wq_sb = wpool.tile([P, KO, DQ], BF16, tag="wq")
    nc.scalar.dma_start(wq_sb[:, 0:4, :], wq[:, 0:4, :])
    wkv_sb = wpool.tile([P, KO, 2 * HD], BF16, tag="wkv")

    # all x tiles prefetched up-front (pool recycling paces the later ones);
    # the first tile arrives in ko quarters interleaved with the weight
    # chunks so the DMA device delivers ko 0..3 of everything first
    xts = {}
    for b in range(B):
        for tb in range(NT):
            xts[(b, tb)] = xt_pool.tile([P, KO, TB], BF16, tag="xt", name=f"xt{b}{tb}")
    src00 = xT[0, :, :, 0:TB]
    nc.scalar.dma_start(xts[(0, 0)][:, 0:4, :], src00[:, 0:4, :])
    nc.scalar.dma_start(wq_sb[:, 4:KO, :], wq[:, 4:KO, :])
    nc.scalar.dma_start(xts[(0, 0)][:, 4:KO, :], src00[:, 4:KO, :])
    nc.scalar.dma_start(wkv_sb[:], wkv[:])
    ident = wpool.tile([P, P], BF16, tag="ident")
    make_identity(nc, ident[:])
    for b in range(B):
        for tb in range(NT):
            if (b, tb) == (0, 0):
                continue
            nc.scalar.dma_start(
                xts[(b, tb)][:], xT[b, :, :, tb * TB : (tb + 1) * TB]
            )

    qts, kkts, vts, v1s = {}, {}, {}, {}
    for b in range(B):
        qts[b] = qt_pool.tile([P, 2, T], BF16, tag="qt", name=f"qt{b}")
        kkts[b] = kkt_pool.tile([P, T], BF16, tag="kkt", name=f"kkt{b}")
        vts[b] = vt_pool.tile([P, T], BF16, tag="vt", name=f"vt{b}")
        v1s[b] = v1_pool.tile([P, NKT, P], BF16, tag="v1", name=f"v1{b}")

    # ---------------- projections ----------------
    for b in range(B):
        qt_sb, kkt_sb, vt_sb, v1_sb = qts[b], kkts[b], vts[b], v1s[b]
        for tb in range(NT):
            xt = xts[(b, tb)]
            q_ps = pp.tile([P, 2, TB], F32, tag="S")
            kv_ps = pp.tile([P, TB], F32, tag="C")
            # for the very first block, run all q matmuls before the kv ones:
            # wq + x alone give PE a full runway while wkv streams in
            if b == 0 and tb == 0:
                for ko in range(KO):
                    st, sp_ = (ko == 0), (ko == KO - 1)
                    nc.tensor.matmul(
                        q_ps[:, 0, :], wq_sb[:, ko, 0:P], xt[:, ko, :],
                        start=st, stop=sp_,
                    )
                    nc.tensor.matmul(
                        q_ps[:, 1, :], wq_sb[:, ko, P:DQ], xt[:, ko, :],
                        start=st, stop=sp_,
                    )
                for ko in range(KO):
                    nc.tensor.matmul(
                        kv_ps[:], wkv_sb[:, ko, :], xt[:, ko, :],
                        start=(ko == 0), stop=(ko == KO - 1),
                    )
            else:
                for ko in range(KO):
                    st, sp_ = (ko == 0), (ko == KO - 1)
                    nc.tensor.matmul(
                        q_ps[:, 0, :], wq_sb[:, ko, 0:P], xt[:, ko, :],
                        start=st, stop=sp_,
                    )
                    nc.tensor.matmul(
                        q_ps[:, 1, :], wq_sb[:, ko, P:DQ], xt[:, ko, :],
                        start=st, stop=sp_,
                    )
                    nc.tensor.matmul(
                        kv_ps[:], wkv_sb[:, ko, :], xt[:, ko, :], start=st, stop=sp_
                    )
            ts_ = slice(tb * TB, (tb + 1) * TB)
            if tb == NT - 1:
                # last block's qt gates the first attention scores -> mul first
                nc.vector.tensor_scalar_mul(qt_sb[:, :, ts_], q_ps[:], SCALE)
                nc.vector.tensor_copy(kkt_sb[0:HD, ts_], kv_ps[0:HD, :])
                nc.vector.tensor_copy(vt_sb[HD:P, ts_], kv_ps[HD:P, :])
            else:
                nc.vector.tensor_copy(kkt_sb[0:HD, ts_], kv_ps[0:HD, :])
                nc.vector.tensor_copy(vt_sb[HD:P, ts_], kv_ps[HD:P, :])
                nc.vector.tensor_scalar_mul(qt_sb[:, :, ts_], q_ps[:], SCALE)
            # duplicate KT to partitions 64..127 (SBUF->SBUF DMA moves partitions)
            nc.sync.dma_start(kkt_sb[HD:P, ts_], kkt_sb[0:HD, ts_])

        # V transpose -> v1[kpos, 0:64]; ones in v1[kpos, 64:128] replicate the
        # softmax denominator across partitions 64..127 of the PV output
        nc.gpsimd.memset(v1_sb[:, :, HD:P], 1.0)
        for kt in range(NKT):
            tr_ps = pp.tile([P, HD], BF16, tag="D")
            nc.tensor.transpose(
                tr_ps[:],
                vt_sb[HD:P, kt * P : (kt + 1) * P],
                ident[HD:P, HD:P],
            )
            nc.vector.tensor_copy(v1_sb[:, kt, 0:HD], tr_ps[:])

    wo_sb = wpool.tile([P, 2, D], BF16, tag="wo")
    nc.scalar.dma_start(wo_sb[:], wo[:])

    # ---------------- attention + output proj ----------------
    # Wo work of the previous block drips into the current kt loop (one chunk
    # per kt iteration) so it fills PE stalls caused by the exp chain.
    pending = []

    def emit_pending(n):
        for _ in range(min(n, len(pending))):
            pending.pop(0)()

    def queue_wo(b, qb, ot_sb, final=False):
        state = {}
        if final:
            # tail flush: 2-bank S-pool tiles (free after the last kt loop),
            # merged ACT/DVE-alternating copies, per-half stores -> the flush
            # runs matmul-dense and the last bytes leave right behind it
            for tt in range(4):
                for nbp in range(2):

                    def chunk2(tt=tt, nbp=nbp):
                        if nbp == 0:
                            state[tt] = stg_pool.tile(
                                [P, D], BF16, tag="stg", name=f"stg{b}{qb}{tt}"
                            )
                        stg = state[tt]
                        wo_ps2 = pp.tile([P, 2, TB], F32, tag="S", name="wo_ps2")
                        for sub in range(2):
                            nb = 2 * nbp + sub
                            for ko in range(2):
                                nc.tensor.matmul(
                                    wo_ps2[:, sub, :],
                                    ot_sb[:, ko, tt * P : (tt + 1) * P],
                                    wo_sb[:, ko, nb * TB : (nb + 1) * TB],
                                    start=(ko == 0),
                                    stop=(ko == 1),
                                )
                        hs = slice(2 * nbp * TB, (2 * nbp + 2) * TB)
                        if (tt * 2 + nbp) % 2 == 0:
                            nc.scalar.activation(stg[:, hs], wo_ps2[:], AF.Copy)
                        else:
                            nc.vector.tensor_copy(stg[:, hs], wo_ps2[:])
                        nc.sync.dma_start(out[b, qb, :, tt, hs], stg[:, hs])

                    pending.append(chunk2)
            return
        for tt in range(4):
            for nb in range(4):

                def chunk(tt=tt, nb=nb):
                    if nb == 0:
                        state[tt] = stg_pool.tile(
                            [P, D], BF16, tag="stg", name=f"stg{b}{qb}{tt}"
                        )
                    stg = state[tt]
                    wo_ps = pp.tile([P, TB], F32, tag="D")
                    for ko in range(2):
                        nc.tensor.matmul(
                            wo_ps[:],
                            ot_sb[:, ko, tt * P : (tt + 1) * P],
                            wo_sb[:, ko, nb * TB : (nb + 1) * TB],
                            start=(ko == 0),
                            stop=(ko == 1),
                        )
                    ns_ = slice(nb * TB, (nb + 1) * TB)
                    nc.vector.tensor_copy(stg[:, ns_], wo_ps[:])
                    if nb == 3:
                        nc.sync.dma_start(out[b, qb, :, tt, :], stg[:])

                pending.append(chunk)

    for b in range(B):
        qt_sb, kkt_sb, v1_sb = qts[b], kkts[b], v1s[b]
        # largest first; the all-diagonal qb=0 sits second-to-last so its long
        # per-tile chains are filled by qb=2's Wo drip, and the run ends on
        # qb=1 whose trailing off-diagonal tiles have short chains
        for qb in (3, 2, 0, 1):
            nkt = 4 * (qb + 1)
            # drip the previous block's Wo chunks evenly across this block's
            # kt iterations so they fill exp-chain PE stalls
            emit_rate = len(pending) / (2.0 * nkt)
            emit_acc = 0.0
            emitted = 0
            ot_sb = o_pool.tile([P, 2, TB], BF16, tag="ot")
            # diagonal blocks first: their long S->exp->mask->PV chains hide
            # behind the off-diagonal stream that follows (the j=0 diagonal
            # covers the full q range, so it can carry start=True). The very
            # first block instead runs off-diagonal first: its diagonal tiles
            # need the just-finished last projection block (qt/kkt cols
            # 1536:2048), while kt=0.. is ready immediately.
            if b == 0 and qb == 3 and len(pending) == 0:
                kt_order = list(range(0, nkt))
            else:
                kt_order = list(range(4 * qb, nkt)) + list(range(0, 4 * qb))
            for pair in range(2):
                o_ps = []
                for i in range(2):
                    o_ps.append(pp.tile([P, TB], F32, tag="C", name=f"o_ps_{i}"))
                for ki, kt in enumerate(kt_order):
                    ks = slice(kt * P, (kt + 1) * P)
                    j = kt - 4 * qb  # >=0 on diagonal blocks
                    cs = slice(128 * j, TB) if j >= 0 else slice(0, TB)
                    w = TB - cs.start
                    qs = slice(qb * TB + cs.start, (qb + 1) * TB)
                    s_ps = pp.tile([P, 2, TB], F32, tag="S")
                    nc.tensor.matmul(
                        s_ps[:, 0, cs],
                        kkt_sb[0:HD, ks],
                        qt_sb[0:HD, pair, qs],
                        start=True,
                        stop=True,
                        tile_position=(0, 0),
                    )
                    nc.tensor.matmul(
                        s_ps[:, 1, cs],
                        kkt_sb[HD:P, ks],
                        qt_sb[HD:P, pair, qs],
                        start=True,
                        stop=True,
                        tile_position=(64, 0),
                    )
                    pt = p_pool.tile([P, 2, TB], BF16, tag="p")
                    nc.scalar.activation(pt[:, :, cs], s_ps[:, :, cs], AF.Exp)
                    if j >= 0:  # diagonal: zero out q < k inside the block
                        nc.gpsimd.affine_select(
                            out=pt[:, :, cs],
                            in_=pt[:, :, cs],
                            compare_op=mybir.AluOpType.is_ge,
                            fill=0.0,
                            base=0,
                            channel_multiplier=-1,
                            pattern=[[0, 2], [1, w]],
                        )
                    st, sp_ = (ki == 0), (ki == nkt - 1)
                    nc.tensor.matmul(
                        o_ps[0][:, cs], v1_sb[:, kt, :], pt[:, 0, cs],
                        start=st, stop=sp_,
                    )
                    nc.tensor.matmul(
                        o_ps[1][:, cs], v1_sb[:, kt, :], pt[:, 1, cs],
                        start=st, stop=sp_,
                    )
                    emit_acc += emit_rate
                    n_emit = int(emit_acc) - emitted
                    emitted += n_emit
                    emit_pending(n_emit)
                if b == B - 1 and qb == 1 and pair == 1:
                    # very last pair: normalize per 128-token chunk straight
                    # from PSUM so the first tail-flush Wo matmul starts after
                    # ~1us instead of after the full-width chain
                    rbs = []
                    for i in range(2):
                        rb = r_pool.tile([HD, TB], F32, tag="rb", name=f"rbf{i}")
                        rbs.append(rb)
                    for tt in range(4):
                        tsl = slice(tt * P, (tt + 1) * P)
                        for i in range(2):
                            nc.vector.reciprocal(
                                rbs[i][:, tsl], o_ps[i][HD:P, tsl]
                            )
                            nc.vector.tensor_mul(
                                ot_sb[i * HD : (i + 1) * HD, pair, tsl],
                                o_ps[i][0:HD, tsl],
                                rbs[i][:, tsl],
                            )
                    continue
                # evacuate PSUM with one ACT copy per head (frees the o_ps
                # banks for the next pair ~1.5us earlier than recip+mul would),
                # then normalize: ot[r] = o_cp[0:64] / denom (rows 64:127)
                for i in range(2):
                    o_cp = r_pool.tile([P, TB], F32, tag="ocp", name=f"ocp{i}")
                    if i == 0:  # both banks evacuate in parallel (DVE + ACT)
                        nc.vector.tensor_copy(o_cp[:], o_ps[i][:])
                    else:
                        nc.scalar.activation(o_cp[:], o_ps[i][:], AF.Copy)
                    rb = r_pool.tile([HD, TB], F32, tag="rb")
                    nc.vector.reciprocal(rb[:], o_cp[HD:P, :])
                    nc.vector.tensor_mul(
                        ot_sb[i * HD : (i + 1) * HD, pair, :],
                        o_cp[0:HD, :],
                        rb[:],
                    )
            queue_wo(b, qb, ot_sb, final=(b == B - 1 and qb == 1))
    emit_pending(len(pending))


_NC_CACHE = {}


def get_nc():
    if "nc" not in _NC_CACHE:
        nc = bacc.Bacc("TRN2", target_bir_lowering=False, debug=False)
        with tile.TileContext(nc) as tc, ExitStack() as ctx:
            build_kernel(ctx, tc)
        nc.compile()
        _NC_CACHE["nc"] = nc
    return _NC_CACHE["nc"]


def make_in_maps(x, Wq, Wk, Wv, Wo):
    import ml_dtypes

    bf = ml_dtypes.bfloat16
    x = np.asarray(x, np.float32)
    # xT[b, p, ko, t] = x[b, t, ko*128 + p]
    xTr = np.ascontiguousarray(
        x.transpose(0, 2, 1).reshape(B, KO, P, T).transpose(0, 2, 1, 3)
    ).astype(bf)
    Wq, Wk, Wv, Wo = (np.asarray(w, np.float32) for w in (Wq, Wk, Wv, Wo))
    in_maps = []
    for g in range(NCORES):
        wq_g = Wq[:, g * DQ : (g + 1) * DQ].reshape(KO, P, DQ).transpose(1, 0, 2)
        wkv_g = (
            np.concatenate(
                [Wk[:, g * HD : (g + 1) * HD], Wv[:, g * HD : (g + 1) * HD]], axis=1
            )
            .reshape(KO, P, 2 * HD)
            .transpose(1, 0, 2)
        )
        wo_g = Wo[g * DQ : (g + 1) * DQ, :].reshape(2, P, D).transpose(1, 0, 2)
        in_maps.append(
            {
                "xT": xTr,
                "wq": np.ascontiguousarray(wq_g).astype(bf),
                "wkv": np.ascontiguousarray(wkv_g).astype(bf),
                "wo": np.ascontiguousarray(wo_g).astype(bf),
            }
        )
    return in_maps


def run(x, Wq, Wk, Wv, Wo, trace=False):
    nc = get_nc()
    in_maps = make_in_maps(x, Wq, Wk, Wv, Wo)
    res = run_bass_kernel_spmd(nc, in_maps, list(range(NCORES)), trace=trace)
    acc = np.zeros((B, NT, P, 4, D), np.float32)
    for r in res.results:
        acc += np.asarray(r["out"], np.float32)
    # [b, qb, p, tt, d] -> [b, qb, tt, p, d] -> [B, T, D]
    full = np.ascontiguousarray(acc.transpose(0, 1, 3, 2, 4)).reshape(B, T, D)
    return full, res


def kernel(x, Wq, Wk, Wv, Wo):
    return run(x, Wq, Wk, Wv, Wo)[0]


# revision 35
# speedup vs baseline: 233.7651x; 1.0151x over previous
"""GQA attention kernel for Trainium2, tensor-parallel across 8 NeuronCores.

Problem: B=2, T=2048, D=2048, H=32 q-heads, G=8 kv-heads (GQA, rep=4), hd=64,
causal softmax attention + output projection, fp32 I/O.

Sharding (one KV group per core):
  core g: Wq[:, g*256:(g+1)*256], Wk/Wv[:, g*64:(g+1)*64], Wo[g*256:(g+1)*256, :]
  Each core computes its 4 heads' attention + partial output projection;
  host sums the 8 partial outputs (row-parallel Wo => partial-sum unshard).

Device dataflow (bf16 activations/weights/partials; fp32 PSUM accumulation):
  QT = wq.T @ xT        [256, T]  (1/8 scale folded into the PSUM->SBUF copy)
  [KT; VT] = wkv.T @ xT [128, T]; KT duplicated to partitions 64..127 so two
  heads' score matmuls run in disjoint PE row groups (contraction = hd = 64).
  Scores for a head-pair land in one 2-bank PSUM tile [128, 2, 512] so exp and
  the diagonal causal mask run once per pair (half the ACT/Pool instructions).
  Diagonal blocks are trapezoid-trimmed: matmul/exp/mask/PV only touch
  columns >= 128*j of the block (j = diagonal sub-tile index), and are
  processed first within a block so their long chains hide behind the
  off-diagonal stream (except the very first block, which needs the last
  projection tile and therefore runs ascending).
  V tiles carry 64 columns of ones -> the PV matmul replicates the softmax
  denominator across partitions 64..127; normalize = reciprocal + mul off an
  ACT/DVE PSUM evacuation copy (frees the o banks for the next head pair).
  partial = OT.T @ wo, staged per 128-token chunk to SBUF as bf16, stored per
  qb/tt block; host upcasts and sums the 8 partials. Wo work for block i is
  dripped into block i+1's kt loop (one chunk per iteration) to fill the PE
  stalls left by the exp chain; the last block flushes through 2-bank PSUM
  tiles with per-half stores so the tail is matmul-dense.
"""

import os
import sys

import numpy as np

for _p in ("/opt/trn_rl_repo", "/root/.axon_site/_ro/trn_rl_repo"):
    if os.path.isdir(_p) and _p not in sys.path:
        sys.path.insert(0, _p)

import concourse.bass as bass  # noqa: E402
import concourse.mybir as mybir  # noqa: E402
import concourse.tile as tile  # noqa: E402
from concourse import bacc  # noqa: E402
from concourse.bass_utils import run_bass_kernel_spmd  # noqa: E402
from concourse.masks import make_identity  # noqa: E402
from contextlib import ExitStack  # noqa: E402

B, T, D = 2, 2048, 2048
G, REP, HD = 8, 4, 64
DQ = REP * HD  # 256 q-dims per core
NCORES = 8
P = 128
TB = 512  # q/t block size
KO = D // P  # 16 contraction subtiles for projections
NT = T // TB  # 4 t-blocks
NKT = T // P  # 16 kpos tiles
F32 = mybir.dt.float32
BF16 = mybir.dt.bfloat16
AF = mybir.ActivationFunctionType
SCALE = 1.0 / 8.0  # 1/sqrt(HD)


def build_kernel(ctx, tc):
    nc = tc.nc
    # host pre-reshapes everything into the exact SBUF layouts
    xT = nc.dram_tensor("xT", [B, P, KO, T], BF16, kind="ExternalInput").ap()
    wq = nc.dram_tensor("wq", [P, KO, DQ], BF16, kind="ExternalInput").ap()
    wkv = nc.dram_tensor("wkv", [P, KO, 2 * HD], BF16, kind="ExternalInput").ap()
    wo = nc.dram_tensor("wo", [P, 2, D], BF16, kind="ExternalInput").ap()
    # out[b, qb, p, tt, :] = partial for token qb*512 + tt*128 + p
    out = nc.dram_tensor("out", [B, NT, P, 4, D], BF16, kind="ExternalOutput").ap()

    wpool = ctx.enter_context(tc.tile_pool(name="w", bufs=1))
    qt_pool = ctx.enter_context(tc.tile_pool(name="qt", bufs=2))
    kkt_pool = ctx.enter_context(tc.tile_pool(name="kkt", bufs=2))
    vt_pool = ctx.enter_context(tc.tile_pool(name="vt", bufs=2))
    v1_pool = ctx.enter_context(tc.tile_pool(name="v1", bufs=2))
    xt_pool = ctx.enter_context(tc.tile_pool(name="xt", bufs=3))
    p_pool = ctx.enter_context(tc.tile_pool(name="p", bufs=6))
    o_pool = ctx.enter_context(tc.tile_pool(name="ot", bufs=2))
    r_pool = ctx.enter_context(tc.tile_pool(name="rcp", bufs=3))
    stg_pool = ctx.enter_context(tc.tile_pool(name="stg", bufs=4))
    pp = ctx.enter_context(tc.tile_pool(name="pp", bufs=2, space="PSUM"))

    # persistent weights; wq/wkv split so the first projection matmuls start
    # after ~1/4 of the weight traffic; wo deferred until after projections
    wq_sb = wpool.tile([P, KO, DQ], BF16, tag="wq")
    nc.scalar.dma_start(wq_sb[:, 0:4, :], wq[:, 0:4, :])
    wkv_sb = wpool.tile([P, KO, 2 * HD], BF16, tag="wkv")

    # all x tiles prefetched up-front (pool recycling paces the later ones);
    # the first tile arrives in ko quarters interleaved with the weight
    # chunks so the DMA device delivers ko 0..3 of everything first
    xts = {}
    for b in range(B):
        for tb in range(NT):
            xts[(b, tb)] = xt_pool.tile([P, KO, TB], BF16, tag="xt", name=f"xt{b}{tb}")
    src00 = xT[0, :, :, 0:TB]
    src01 = xT[0, :, :, TB : 2 * TB]
    # interleave wq/x thirds then wkv/x halves: the DMA device feeds PE at
    # just above its consumption rate through the whole prologue
    nc.scalar.dma_start(xts[(0, 0)][:, 0:4, :], src00[:, 0:4, :])
    nc.scalar.dma_start(wq_sb[:, 4:10, :], wq[:, 4:10, :])
    nc.scalar.dma_start(xts[(0, 0)][:, 4:10, :], src00[:, 4:10, :])
    nc.scalar.dma_start(wq_sb[:, 10:KO, :], wq[:, 10:KO, :])
    nc.scalar.dma_start(xts[(0, 0)][:, 10:KO, :], src00[:, 10:KO, :])
    nc.scalar.dma_start(wkv_sb[:, 0:8, :], wkv[:, 0:8, :])
    nc.scalar.dma_start(xts[(0, 1)][:, 0:8, :], src01[:, 0:8, :])
    nc.scalar.dma_start(wkv_sb[:, 8:KO, :], wkv[:, 8:KO, :])
    nc.scalar.dma_start(xts[(0, 1)][:, 8:KO, :], src01[:, 8:KO, :])
    ident = wpool.tile([P, P], BF16, tag="ident")
    make_identity(nc, ident[:])
    for b in range(B):
        for tb in range(NT):
            if (b, tb) in ((0, 0), (0, 1)):
                continue
            nc.scalar.dma_start(
                xts[(b, tb)][:], xT[b, :, :, tb * TB : (tb + 1) * TB]
            )

    qts, kkts, vts, v1s = {}, {}, {}, {}
    for b in range(B):
        qts[b] = qt_pool.tile([P, 2, T], BF16, tag="qt", name=f"qt{b}")
        kkts[b] = kkt_pool.tile([P, T], BF16, tag="kkt", name=f"kkt{b}")
        vts[b] = vt_pool.tile([P, T], BF16, tag="vt", name=f"vt{b}")
        v1s[b] = v1_pool.tile([P, NKT, P], BF16, tag="v1", name=f"v1{b}")

    # ---------------- projections ----------------
    for b in range(B):
        qt_sb, kkt_sb, vt_sb, v1_sb = qts[b], kkts[b], vts[b], v1s[b]
        for tb in range(NT):
            xt = xts[(b, tb)]
            q_ps = pp.tile([P, 2, TB], F32, tag="S")
            kv_ps = pp.tile([P, TB], F32, tag="C")
            # for the very first block, run all q matmuls before the kv ones:
            # wq + x alone give PE a full runway while wkv streams in
            if b == 0 and tb == 0:
                for ko in range(KO):
                    st, sp_ = (ko == 0), (ko == KO - 1)
                    nc.tensor.matmul(
                        q_ps[:, 0, :], wq_sb[:, ko, 0:P], xt[:, ko, :],
                        start=st, stop=sp_,
                    )
                    nc.tensor.matmul(
                        q_ps[:, 1, :], wq_sb[:, ko, P:DQ], xt[:, ko, :],
                        start=st, stop=sp_,
                    )
                for ko in range(KO):
                    nc.tensor.matmul(
                        kv_ps[:], wkv_sb[:, ko, :], xt[:, ko, :],
                        start=(ko == 0), stop=(ko == KO - 1),
                    )
            else:
                for ko in range(KO):
                    st, sp_ = (ko == 0), (ko == KO - 1)
                    nc.tensor.matmul(
                        q_ps[:, 0, :], wq_sb[:, ko, 0:P], xt[:, ko, :],
                        start=st, stop=sp_,
                    )
                    nc.tensor.matmul(
                        q_ps[:, 1, :], wq_sb[:, ko, P:DQ], xt[:, ko, :],
                        start=st, stop=sp_,
                    )
                    nc.tensor.matmul(
                        kv_ps[:], wkv_sb[:, ko, :], xt[:, ko, :], start=st, stop=sp_
                    )
            ts_ = slice(tb * TB, (tb + 1) * TB)
            if tb == NT - 1:
                # last block's qt gates the first attention scores -> mul first
                nc.vector.tensor_scalar_mul(qt_sb[:, :, ts_], q_ps[:], SCALE)
                nc.vector.tensor_copy(kkt_sb[0:HD, ts_], kv_ps[0:HD, :])
                nc.vector.tensor_copy(vt_sb[HD:P, ts_], kv_ps[HD:P, :])
            else:
                nc.vector.tensor_copy(kkt_sb[0:HD, ts_], kv_ps[0:HD, :])
                nc.vector.tensor_copy(vt_sb[HD:P, ts_], kv_ps[HD:P, :])
                nc.vector.tensor_scalar_mul(qt_sb[:, :, ts_], q_ps[:], SCALE)
            # duplicate KT to partitions 64..127 (SBUF->SBUF DMA moves partitions)
            nc.sync.dma_start(kkt_sb[HD:P, ts_], kkt_sb[0:HD, ts_])

        # V transpose -> v1[kpos, 0:64]; ones in v1[kpos, 64:128] replicate the
        # softmax denominator across partitions 64..127 of the PV output
        nc.gpsimd.memset(v1_sb[:, :, HD:P], 1.0)
        for kt in range(NKT):
            tr_ps = pp.tile([P, HD], BF16, tag="D")
            nc.tensor.transpose(
                tr_ps[:],
                vt_sb[HD:P, kt * P : (kt + 1) * P],
                ident[HD:P, HD:P],
            )
            nc.vector.tensor_copy(v1_sb[:, kt, 0:HD], tr_ps[:])

    wo_sb = wpool.tile([P, 2, D], BF16, tag="wo")
    nc.scalar.dma_start(wo_sb[:], wo[:])

    # ---------------- attention + output proj ----------------
    # Wo work of the previous block drips into the current kt loop (one chunk
    # per kt iteration) so it fills PE stalls caused by the exp chain.
    pending = []

    def emit_pending(n):
        for _ in range(min(n, len(pending))):
            pending.pop(0)()

    def queue_wo(b, qb, ot_sb, final=False):
        state = {}
        if final:
            # tail flush: 2-bank S-pool tiles (free after the last kt loop),
            # merged ACT/DVE-alternating copies, per-half stores -> the flush
            # runs matmul-dense and the last bytes leave right behind it
            for tt in range(4):
                for nbp in range(2):

                    def chunk2(tt=tt, nbp=nbp):
                        if nbp == 0:
                            state[tt] = stg_pool.tile(
                                [P, D], BF16, tag="stg", name=f"stg{b}{qb}{tt}"
                            )
                        stg = state[tt]
                        wo_ps2 = pp.tile([P, 2, TB], F32, tag="S", name="wo_ps2")
                        for sub in range(2):
                            nb = 2 * nbp + sub
                            for ko in range(2):
                                nc.tensor.matmul(
                                    wo_ps2[:, sub, :],
                                    ot_sb[:, ko, tt * P : (tt + 1) * P],
                                    wo_sb[:, ko, nb * TB : (nb + 1) * TB],
                                    start=(ko == 0),
                                    stop=(ko == 1),
                                )
                        hs = slice(2 * nbp * TB, (2 * nbp + 2) * TB)
                        if (tt * 2 + nbp) % 2 == 0:
                            nc.scalar.activation(stg[:, hs], wo_ps2[:], AF.Copy)
                        else:
                            nc.vector.tensor_copy(stg[:, hs], wo_ps2[:])
                        nc.sync.dma_start(out[b, qb, :, tt, hs], stg[:, hs])

                    pending.append(chunk2)
            return
        for tt in range(4):
            for nb in range(4):

                def chunk(tt=tt, nb=nb):
                    if nb == 0:
                        state[tt] = stg_pool.tile(
                            [P, D], BF16, tag="stg", name=f"stg{b}{qb}{tt}"
                        )
                    stg = state[tt]
                    wo_ps = pp.tile([P, TB], F32, tag="D")
                    for ko in range(2):
                        nc.tensor.matmul(
                            wo_ps[:],
                            ot_sb[:, ko, tt * P : (tt + 1) * P],
                            wo_sb[:, ko, nb * TB : (nb + 1) * TB],
                            start=(ko == 0),
                            stop=(ko == 1),
                        )
                    ns_ = slice(nb * TB, (nb + 1) * TB)
                    nc.vector.tensor_copy(stg[:, ns_], wo_ps[:])
                    if nb == 3:
                        nc.sync.dma_start(out[b, qb, :, tt, :], stg[:])

                pending.append(chunk)

    for b in range(B):
        qt_sb, kkt_sb, v1_sb = qts[b], kkts[b], v1s[b]
        # largest first; the all-diagonal qb=0 sits second-to-last so its long
        # per-tile chains are filled by qb=2's Wo drip, and the run ends on
        # qb=1 whose trailing off-diagonal tiles have short chains
        for qb in (3, 2, 0, 1):
            nkt = 4 * (qb + 1)
            # drip the previous block's Wo chunks evenly across this block's
            # kt iterations so they fill exp-chain PE stalls
            emit_rate = len(pending) / (2.0 * nkt)
            emit_acc = 0.0
            emitted = 0
            ot_sb = o_pool.tile([P, 2, TB], BF16, tag="ot")
            # diagonal blocks first: their long S->exp->mask->PV chains hide
            # behind the off-diagonal stream that follows (the j=0 diagonal
            # covers the full q range, so it can carry start=True). The very
            # first block instead runs off-diagonal first: its diagonal tiles
            # need the just-finished last projection block (qt/kkt cols
            # 1536:2048), while kt=0.. is ready immediately.
            if b == 0 and qb == 3 and len(pending) == 0:
                kt_order = list(range(0, nkt))
            else:
                kt_order = list(range(4 * qb, nkt)) + list(range(0, 4 * qb))
            for pair in range(2):
                o_ps = []
                for i in range(2):
                    o_ps.append(pp.tile([P, TB], F32, tag="C", name=f"o_ps_{i}"))
                for ki, kt in enumerate(kt_order):
                    ks = slice(kt * P, (kt + 1) * P)
                    j = kt - 4 * qb  # >=0 on diagonal blocks
                    cs = slice(128 * j, TB) if j >= 0 else slice(0, TB)
                    w = TB - cs.start
                    qs = slice(qb * TB + cs.start, (qb + 1) * TB)
                    s_ps = pp.tile([P, 2, TB], F32, tag="S")
                    nc.tensor.matmul(
                        s_ps[:, 0, cs],
                        kkt_sb[0:HD, ks],
                        qt_sb[0:HD, pair, qs],
                        start=True,
                        stop=True,
                        tile_position=(0, 0),
                    )
                    nc.tensor.matmul(
                        s_ps[:, 1, cs],
                        kkt_sb[HD:P, ks],
                        qt_sb[HD:P, pair, qs],
                        start=True,
                        stop=True,
                        tile_position=(64, 0),
                    )
                    pt = p_pool.tile([P, 2, TB], BF16, tag="p")
                    nc.scalar.activation(pt[:, :, cs], s_ps[:, :, cs], AF.Exp)
                    if j >= 0:  # diagonal: zero out q < k inside the block
                        nc.gpsimd.affine_select(
                            out=pt[:, :, cs],
                            in_=pt[:, :, cs],
                            compare_op=mybir.AluOpType.is_ge,
                            fill=0.0,
                            base=0,
                            channel_multiplier=-1,
                            pattern=[[0, 2], [1, w]],
                        )
                    st, sp_ = (ki == 0), (ki == nkt - 1)
                    nc.tensor.matmul(
                        o_ps[0][:, cs], v1_sb[:, kt, :], pt[:, 0, cs],
                        start=st, stop=sp_,
                    )
                    nc.tensor.matmul(
                        o_ps[1][:, cs], v1_sb[:, kt, :], pt[:, 1, cs],
                        start=st, stop=sp_,
                    )
                    emit_acc += emit_rate
                    n_emit = int(emit_acc) - emitted
                    emitted += n_emit
                    emit_pending(n_emit)
                if b == B - 1 and qb == 1 and pair == 1:
                    # very last pair: normalize per 128-token chunk straight
                    # from PSUM so the first tail-flush Wo matmul starts after
                    # ~1us instead of after the full-width chain
                    rbs = []
                    for i in range(2):
                        rb = r_pool.tile([HD, TB], F32, tag="rb", name=f"rbf{i}")
                        rbs.append(rb)
                    for tt in range(4):
                        tsl = slice(tt * P, (tt + 1) * P)
                        for i in range(2):
                            nc.vector.reciprocal(
                                rbs[i][:, tsl], o_ps[i][HD:P, tsl]
                            )
                            nc.vector.tensor_mul(
                                ot_sb[i * HD : (i + 1) * HD, pair, tsl],
                                o_ps[i][0:HD, tsl],
                                rbs[i][:, tsl],
                            )
                    continue
                # evacuate PSUM with one ACT copy per head (frees the o_ps
                # banks for the next pair ~1.5us earlier than recip+mul would),
                # then normalize: ot[r] = o_cp[0:64] / denom (rows 64:127)
                for i in range(2):
                    o_cp = r_pool.tile([P, TB], F32, tag="ocp", name=f"ocp{i}")
                    if i == 0:  # both banks evacuate in parallel (DVE + ACT)
                        nc.vector.tensor_copy(o_cp[:], o_ps[i][:])
                    else:
                        nc.scalar.activation(o_cp[:], o_ps[i][:], AF.Copy)
                    rb = r_pool.tile([HD, TB], F32, tag="rb")
                    nc.vector.reciprocal(rb[:], o_cp[HD:P, :])
                    nc.vector.tensor_mul(
                        ot_sb[i * HD : (i + 1) * HD, pair, :],
                        o_cp[0:HD, :],
                        rb[:],
                    )
            queue_wo(b, qb, ot_sb, final=(b == B - 1 and qb == 1))
    emit_pending(len(pending))


_NC_CACHE = {}


def get_nc():
    if "nc" not in _NC_CACHE:
        nc = bacc.Bacc("TRN2", target_bir_lowering=False, debug=False)
        with tile.TileContext(nc) as tc, ExitStack() as ctx:
            build_kernel(ctx, tc)
        nc.compile()
        _NC_CACHE["nc"] = nc
    return _NC_CACHE["nc"]


def make_in_maps(x, Wq, Wk, Wv, Wo):
    import ml_dtypes

    bf = ml_dtypes.bfloat16
    x = np.asarray(x, np.float32)
    # xT[b, p, ko, t] = x[b, t, ko*128 + p]
    xTr = np.ascontiguousarray(
        x.transpose(0, 2, 1).reshape(B, KO, P, T).transpose(0, 2, 1, 3)
    ).astype(bf)
    Wq, Wk, Wv, Wo = (np.asarray(w, np.float32) for w in (Wq, Wk, Wv, Wo))
    in_maps = []
    for g in range(NCORES):
        wq_g = Wq[:, g * DQ : (g + 1) * DQ].reshape(KO, P, DQ).transpose(1, 0, 2)
        wkv_g = (
            np.concatenate(
                [Wk[:, g * HD : (g + 1) * HD], Wv[:, g * HD : (g + 1) * HD]], axis=1
            )
            .reshape(KO, P, 2 * HD)
            .transpose(1, 0, 2)
        )
        wo_g = Wo[g * DQ : (g + 1) * DQ, :].reshape(2, P, D).transpose(1, 0, 2)
        in_maps.append(
            {
                "xT": xTr,
                "wq": np.ascontiguousarray(wq_g).astype(bf),
                "wkv": np.ascontiguousarray(wkv_g).astype(bf),
                "wo": np.ascontiguousarray(wo_g).astype(bf),
            }
        )
    return in_maps


def run(x, Wq, Wk, Wv, Wo, trace=False):
    nc = get_nc()
    in_maps = make_in_maps(x, Wq, Wk, Wv, Wo)
    res = run_bass_kernel_spmd(nc, in_maps, list(range(NCORES)), trace=trace)
    acc = np.zeros((B, NT, P, 4, D), np.float32)
    for r in res.results:
        acc += np.asarray(r["out"], np.float32)
    # [b, qb, p, tt, d] -> [b, qb, tt, p, d] -> [B, T, D]
    full = np.ascontiguousarray(acc.transpose(0, 1, 3, 2, 4)).reshape(B, T, D)
    return full, res


def kernel(x, Wq, Wk, Wv, Wo):
    return run(x, Wq, Wk, Wv, Wo)[0]
